# revision 1
# baseline (speedup 1.0000x reference)
"""MenuLoss Trainium2 kernel.

Math: the loss needs, per batch b, cal[b] = (1/700)*sum_j amt_bj * p(x_bj) for two
evals (true ids continuous, pred ids rounded), where p is a degree-446 Chebyshev
series.  Fold p into a bilinear form p(x) = sum_{a<22, r<21} G[a,r]*T_a(y)*T_r(x),
y = T_21(x) (exact: 22*21=462 >= 447; G solved on host in float64 from the runtime
coeffs).  On device, build the 43 basis functions per element with Chebyshev
recurrence ladders (even indices via ACT Square: T_2m = 2*T_m^2 - 1; odd via DVE
double-step: T_{r+2} = 2*T_2*T_r - T_{r-2}), fold amt into the T_a(y) side, and
contract over elements with TensorE matmuls accumulating per-batch Grams in PSUM.
A signed G-matrix contraction then yields calT[b]-calP[b] directly; penalties are
fused elementwise maps with accumulated reductions.  8-way batch data parallel,
per-core scalar partials summed on host.
"""
import functools
import sys
import types
import numpy as np

# this container's axon build lacks the NTFF profile hook module; stub it so
# run_bass_kernel_spmd(trace=True) degrades to an untraced run instead of dying
if "antenv.axon_hooks" not in sys.modules:
    _m = types.ModuleType("antenv.axon_hooks")
    _m.get_axon_ntff_profile_hook = lambda: None
    sys.modules["antenv.axon_hooks"] = _m

import concourse.bacc as bacc
import concourse.bass as bass
import concourse.mybir as mybir
import concourse.tile as tile
from concourse.bass_utils import run_bass_kernel_spmd

AFT = mybir.ActivationFunctionType
ALU = mybir.AluOpType
FP32 = mybir.dt.float32

N_CORES = 8
B, J = 512, 7 * 16 * 64          # 512 batches, 7168 elements/batch
BC = B // N_CORES                # 64 batches per core
CHUNKS = J // 128                # 56 contraction chunks per batch
K, A = 21, 22                    # p(x) = sum G[a,r] T_a(T_K(x)) T_r(x)
NB = 2 * K                       # 42 combined U cols (U_T | U_P); W rows 44
SL = 8                           # batches per slice
NSL = BC // SL                   # 8 slices
C = SL * CHUNKS                  # 448 columns per slice
R2 = np.sqrt(2.0).astype(np.float32) if False else float(np.sqrt(2.0))


def _shift(i):
    # device basis stores T_i + 1 for even i >= 4 (saves the -1 pass)
    return 1.0 if (i >= 4 and i % 2 == 0) else 0.0


def _fold_G(coeffs: np.ndarray) -> np.ndarray:
    """Solve G[A,K] s.t. sum G[a,r] (T_a(T_K(x))+s_a)(T_r(x)+s_r) == chebval."""
    NN = A * K
    M = np.zeros((NN, NN), np.float64)
    for a in range(A):
        sa = _shift(a)
        for r in range(K):
            sr = _shift(r)
            row = a * K + r
            M[row, a * K + r] += 0.5
            M[row, abs(a * K - r)] += 0.5
            M[row, a * K] += sr
            M[row, r] += sa * 1.0 if a > 0 else sa  # T_r term
            M[row, 0] += sa * sr
    c = np.zeros(NN, np.float64)
    c[: len(coeffs)] = coeffs
    g = np.linalg.solve(M.T, c)
    return g.reshape(A, K)


def _build_ladder(nc, bias_r2, tmp_pool, UU, ubase, nb, seed_kind, ids=None, y=None):
    """Write T_0..T_{nb-1} into UU[:, ubase+r, :] (basis-major [128, *, C]).

    seed_kind 'x': seeds from ids tile (x = ids/111 - 1); returns y=T_21 tile
    seed_kind 'y': seeds from given y tile.
    Returns the T_K tile for seed_kind 'x' (to seed the second level), else None.
    """
    sl = lambda r: UU[:, :, ubase + r]
    q = tmp_pool.tile([128, C], FP32, tag="lad_q")
    u = tmp_pool.tile([128, C], FP32, tag="lad_u")
    m = tmp_pool.tile([128, C], FP32, tag="lad_m")
    if seed_kind == "x":
        # T1 = ids/111 - 1 ; 2x^2 via ACT Square(sqrt2/111 * ids - sqrt2)
        nc.vector.tensor_scalar(sl(1), ids, 1.0 / 111.0, 1.0, ALU.mult, ALU.subtract)
        nc.scalar.activation(q[:], ids, AFT.Square, scale=R2 / 111.0, bias=bias_r2)
        s2 = tmp_pool.tile([128, C], FP32, tag="lad_s2")
        nc.vector.tensor_scalar(s2[:], ids, 2.0 / 111.0, 2.0, ALU.mult, ALU.subtract)
    else:
        nc.scalar.copy(sl(1), y)
        nc.scalar.activation(q[:], y, AFT.Square, scale=R2, bias=0.0)
        s2 = tmp_pool.tile([128, C], FP32, tag="lad_s2")
        nc.vector.tensor_scalar_mul(s2[:], y, 2.0)
    nc.gpsimd.memset(sl(0), 1.0)
    nc.vector.tensor_scalar_sub(sl(2), q[:], 1.0)          # T2 = 2x^2-1
    nc.vector.tensor_scalar(u[:], q[:], 2.0, 2.0, ALU.mult, ALU.subtract)  # u=2*T2
    # T3 = 2x*T2 - x
    nc.vector.tensor_tensor(m[:], s2[:], sl(2), ALU.mult)
    nc.vector.scalar_tensor_tensor(sl(3), m[:], 1.0, sl(1), ALU.mult, ALU.subtract)
    # T4 (shifted: slot = 2*T2^2 = T4+1)
    nc.scalar.activation(sl(4), sl(2), AFT.Square, scale=R2, bias=0.0)
    # odd chain on DVE: T_r = u*T_{r-2} - T_{r-4}
    for r in range(5, nb, 2):
        nc.vector.tensor_tensor(m[:], u[:], sl(r - 2), ALU.mult)
        nc.vector.scalar_tensor_tensor(sl(r), m[:], 1.0, sl(r - 4), ALU.mult,
                                       ALU.subtract)
    # evens >= 6: shifted squares (inputs with even m>=4 are shifted -> bias)
    for r in range(6, nb, 2):
        m2 = r // 2
        b = bias_r2 if (m2 >= 4 and m2 % 2 == 0) else 0.0
        nc.scalar.activation(sl(r), sl(m2), AFT.Square, scale=R2, bias=b)
    if seed_kind == "x":
        # y = T_21 = u*T_19 - T_17
        yt = tmp_pool.tile([128, C], FP32, tag="lad_y")
        nc.vector.tensor_tensor(m[:], u[:], sl(K - 2), ALU.mult)
        nc.vector.scalar_tensor_tensor(yt[:], m[:], 1.0, sl(K - 4), ALU.mult,
                                       ALU.subtract)
        return yt
    return None


def _build(slices=NSL):
    nc = bacc.Bacc("TRN2", target_bir_lowering=False, debug=False, num_devices=1)
    yp = nc.dram_tensor("yp", [BC, J, 2], FP32, kind="ExternalInput")
    yt = nc.dram_tensor("yt", [BC, J, 2], FP32, kind="ExternalInput")
    # signed/scaled G layout [44, SL*43]: rows 0..21 (+G/700) hit the T-eval
    # block (cols b*43+r, r<21); rows 22..43 (-G/700) hit P-block (cols 21+r).
    gc = nc.dram_tensor("gc", [2 * A, SL * NB], FP32, kind="ExternalInput")
    out3 = nc.dram_tensor("out3", [1, 4], FP32, kind="ExternalOutput")

    bias_np = np.broadcast_to(np.array([-np.sqrt(2.0), -222.0], np.float32),
                              (128, 2)).copy()
    bias_dram = nc.inline_tensor(bias_np, name="bias_const")
    yp_r = yp.ap().rearrange("b (c p) t -> p (b c) t", p=128)
    yt_r = yt.ap().rearrange("b (c p) t -> p (b c) t", p=128)

    with tile.TileContext(nc) as tc:
        with (
            tc.tile_pool(name="data", bufs=2) as data_pool,
            tc.tile_pool(name="basis", bufs=1) as basis_pool,
            tc.tile_pool(name="tmp", bufs=1) as tmp_pool,
            tc.tile_pool(name="small", bufs=1) as small_pool,
            tc.tile_pool(name="psum", bufs=2, space="PSUM") as psum_pool,
            tc.tile_pool(name="psc", bufs=2, space="PSUM") as psc_pool,
        ):
            gct = small_pool.tile([2 * A, SL * NB], FP32)
            nc.sync.dma_start(gct[:], gc.ap())
            bias_t = small_pool.tile([128, 2], FP32)
            nc.sync.dma_start(bias_t[:], bias_dram.ap())
            ones44 = small_pool.tile([2 * A, 1], FP32)
            nc.gpsimd.memset(ones44[:], 1.0)
            ones128 = small_pool.tile([128, 1], FP32)
            nc.gpsimd.memset(ones128[:], 1.0)
            diffs = small_pool.tile([1, BC], FP32)
            pen_parts = small_pool.tile([128, NSL], FP32)
            ir_parts = small_pool.tile([128, NSL], FP32)

            for s in range(slices):
                cs = slice(s * C, (s + 1) * C)
                dP = data_pool.tile([128, C, 2], FP32, tag="dP")
                dT = data_pool.tile([128, C, 2], FP32, tag="dT")
                nc.sync.dma_start(dP[:], yp_r[:, cs, :])
                nc.sync.dma_start(dT[:], yt_r[:, cs, :])
                idsP, amtP = dP[:, :, 0], dP[:, :, 1]
                idsT, amtT = dT[:, :, 0], dT[:, :, 1]

                # round pred ids (cast rte matches jnp.round)
                ki = tmp_pool.tile([128, C], mybir.dt.int32, tag="lad_q")
                kf = tmp_pool.tile([128, C], FP32, tag="kf")
                nc.vector.tensor_copy(ki[:], idsP)
                nc.vector.tensor_copy(kf[:], ki[:])

                # penalties on pred slice (raw ids/amt)
                t_i = tmp_pool.tile([128, C], FP32, tag="lad_u")
                t_a = tmp_pool.tile([128, C], FP32, tag="lad_m")
                pm = tmp_pool.tile([128, C], FP32, tag="lad_s2")
                nc.scalar.activation(t_i[:], idsP, AFT.Tanh, scale=4.0)
                nc.scalar.activation(t_a[:], amtP, AFT.Tanh, scale=4.0)
                nc.gpsimd.tensor_tensor(pm[:], t_i[:], t_a[:], ALU.mult)
                nc.vector.scalar_tensor_tensor(pm[:], pm[:], -2.0, t_i[:],
                                               ALU.mult, ALU.add)
                nc.vector.scalar_tensor_tensor(
                    pm[:], pm[:], 1.0, t_a[:], ALU.mult, ALU.add,
                    accum_out=pen_parts[:, s:s + 1])
                ir_t = tmp_pool.tile([128, C], FP32, tag="lad_y")
                nc.scalar.activation(ir_t[:], idsP, AFT.Relu, bias=bias_t[:, 1:2],
                                     accum_out=ir_parts[:, s:s + 1])

                # basis tiles: UU [128, 43, C] (U_T 0..20 | U_P 21..41+1),
                # WW [128, 44, C] (amt*V: T rows 0..21 | P rows 22..43)
                UU = basis_pool.tile([128, C, NB], FP32, tag="UU")
                WW = basis_pool.tile([128, C, 2 * A], FP32, tag="WW")

                for (ids_ap, amt_ap, ub, wb) in (
                    (idsT, amtT, 0, 0),
                    (kf[:], amtP, K, A),
                ):
                    y_t = _build_ladder(nc, bias_t[:, 0:1], tmp_pool, UU, ub, K,
                                        "x", ids=ids_ap)
                    _build_ladder(nc, bias_t[:, 0:1], tmp_pool, WW, wb, A, "y", y=y_t[:])
                    # fold amt into V rows in place (split DVE / gpsimd)
                    for a in range(A):
                        eng = nc.gpsimd if a < 16 else nc.vector
                        eng.tensor_tensor(WW[:, :, wb + a], WW[:, :, wb + a], amt_ap,
                                          ALU.mult)

                # per-batch Grams: psum [44, SL*43], accumulate over 56 chunks
                ps = psum_pool.tile([2 * A, SL * NB], FP32, tag="gram")
                for b in range(SL):
                    for c in range(CHUNKS):
                        j = b * CHUNKS + c
                        nc.tensor.matmul(
                            ps[:, b * NB:(b + 1) * NB],
                            WW[:, j, :], UU[:, j, :],
                            start=(c == 0), stop=(c == CHUNKS - 1))
                # contract with signed G: diffs[b] = calT - calP
                gs = tmp_pool.tile([2 * A, SL * NB], FP32, tag="gs")
                nc.vector.scalar_tensor_tensor(gs[:], ps[:], 1.0, gct[:],
                                               ALU.mult, ALU.mult)
                ps2 = psc_pool.tile([1, SL * NB], FP32, tag="colsum")
                nc.tensor.matmul(ps2[:], ones44[:], gs[:], start=True, stop=True)
                sall = tmp_pool.tile([1, SL * NB], FP32, tag="sall")
                nc.scalar.copy(sall[:], ps2[:])
                nc.vector.tensor_reduce(
                    diffs[:, s * SL:(s + 1) * SL],
                    sall[:].rearrange("p (b n) -> p b n", n=NB),
                    mybir.AxisListType.X, ALU.add)

            # final: sum_b diffs^2, penalty partition-sums
            dsq = small_pool.tile([1, BC], FP32)
            nc.scalar.activation(dsq[:], diffs[:], AFT.Square)
            v0 = small_pool.tile([1, 1], FP32)
            nc.vector.tensor_reduce(v0[:], dsq[:], mybir.AxisListType.X, ALU.add)
            pen_red = small_pool.tile([128, 2], FP32)
            nc.vector.tensor_reduce(pen_red[:, 0:1], pen_parts[:],
                                    mybir.AxisListType.X, ALU.add)
            nc.vector.tensor_reduce(pen_red[:, 1:2], ir_parts[:],
                                    mybir.AxisListType.X, ALU.add)
            ps3 = psc_pool.tile([1, 2], FP32, tag="pen")
            nc.tensor.matmul(ps3[:], ones128[:], pen_red[:], start=True, stop=True)
            ot = small_pool.tile([1, 4], FP32)
            nc.vector.tensor_copy(ot[:, 0:1], v0[:])
            nc.vector.tensor_copy(ot[:, 1:3], ps3[:])
            nc.gpsimd.memset(ot[:, 3:4], 0.0)
            nc.sync.dma_start(out3.ap(), ot[:])
    nc.compile()
    return nc


@functools.lru_cache(maxsize=2)
def _compiled():
    return _build()


def kernel(y_pred: np.ndarray, y: np.ndarray, calories_coeffs: np.ndarray,
           _trace: bool = False):
    G = _fold_G(np.asarray(calories_coeffs, np.float64))
    gc = np.zeros((2 * A, SL * NB), np.float32)
    for b in range(SL):
        gc[:A, b * NB:b * NB + K] = (G / 700.0).astype(np.float32)
        gc[A:, b * NB + K:b * NB + 2 * K] = (-G / 700.0).astype(np.float32)

    ypf = np.ascontiguousarray(y_pred.reshape(B, J, 2), np.float32)
    ytf = np.ascontiguousarray(y.reshape(B, J, 2), np.float32)
    in_maps = []
    for i in range(N_CORES):
        in_maps.append({
            "yp": ypf[i * BC:(i + 1) * BC],
            "yt": ytf[i * BC:(i + 1) * BC],
            "gc": gc,
        })
    nc = _compiled()
    res = run_bass_kernel_spmd(nc, in_maps, list(range(N_CORES)), trace=_trace)
    parts = np.stack([r["out3"][0] for r in res.results])  # [8, 4]
    tot = parts.sum(axis=0)
    loss = (tot[0] + tot[1] + tot[2]) / float(B)
    out = np.float32(loss)
    if _trace:
        return out, res
    return out



# revision 4
# speedup vs baseline: 1.0554x; 1.0554x over previous
"""MenuLoss Trainium2 kernel.

Math: the loss needs, per batch b, cal[b] = (1/700)*sum_j amt_bj * p(x_bj) for two
evals (true ids continuous, pred ids rounded), where p is a degree-446 Chebyshev
series.  Fold p into a bilinear form p(x) = sum_{a<22, r<21} G[a,r]*T_a(y)*T_r(x),
y = T_21(x) (exact: 22*21=462 >= 447; G solved on host in float64 from the runtime
coeffs).  On device, build the 43 basis functions per element with Chebyshev
recurrence ladders (even indices via ACT Square: T_2m = 2*T_m^2 - 1; odd via DVE
double-step: T_{r+2} = 2*T_2*T_r - T_{r-2}), fold amt into the T_a(y) side, and
contract over elements with TensorE matmuls accumulating per-batch Grams in PSUM.
A signed G-matrix contraction then yields calT[b]-calP[b] directly; penalties are
fused elementwise maps with accumulated reductions.  8-way batch data parallel,
per-core scalar partials summed on host.
"""
import functools
import sys
import types
import numpy as np

# this container's axon build lacks the NTFF profile hook module; stub it so
# run_bass_kernel_spmd(trace=True) degrades to an untraced run instead of dying
if "antenv.axon_hooks" not in sys.modules:
    _m = types.ModuleType("antenv.axon_hooks")
    _m.get_axon_ntff_profile_hook = lambda: None
    sys.modules["antenv.axon_hooks"] = _m

import concourse.bacc as bacc
import concourse.bass as bass
import concourse.mybir as mybir
import concourse.tile as tile
from concourse.bass_utils import run_bass_kernel_spmd

AFT = mybir.ActivationFunctionType
ALU = mybir.AluOpType
FP32 = mybir.dt.float32

N_CORES = 8
B, J = 512, 7 * 16 * 64          # 512 batches, 7168 elements/batch
BC = B // N_CORES                # 64 batches per core
CHUNKS = J // 128                # 56 contraction chunks per batch
K, A = 21, 22                    # p(x) = sum G[a,r] T_a(T_K(x)) T_r(x)
NB = 2 * K                       # 42 combined U cols (U_T | U_P); W rows 44
SL = 8                           # batches per slice
NSL = BC // SL                   # 8 slices
C = SL * CHUNKS                  # 448 columns per slice
R2 = np.sqrt(2.0).astype(np.float32) if False else float(np.sqrt(2.0))


def _shift(i):
    # device basis stores T_i + 1 for even i >= 4 (saves the -1 pass)
    return 1.0 if (i >= 4 and i % 2 == 0) else 0.0


def _fold_G(coeffs: np.ndarray) -> np.ndarray:
    """Solve G[A,K] s.t. sum G[a,r] (T_a(T_K(x))+s_a)(T_r(x)+s_r) == chebval."""
    NN = A * K
    M = np.zeros((NN, NN), np.float64)
    for a in range(A):
        sa = _shift(a)
        for r in range(K):
            sr = _shift(r)
            row = a * K + r
            M[row, a * K + r] += 0.5
            M[row, abs(a * K - r)] += 0.5
            M[row, a * K] += sr
            M[row, r] += sa * 1.0 if a > 0 else sa  # T_r term
            M[row, 0] += sa * sr
    c = np.zeros(NN, np.float64)
    c[: len(coeffs)] = coeffs
    g = np.linalg.solve(M.T, c)
    return g.reshape(A, K)


def _build_ladder(nc, bias_r2, tmp_pool, UU, ubase, nb, seed_kind, ids=None, y=None):
    """Write T_0..T_{nb-1} into UU[:, ubase+r, :] (basis-major [128, *, C]).

    seed_kind 'x': seeds from ids tile (x = ids/111 - 1); returns y=T_21 tile
    seed_kind 'y': seeds from given y tile.
    Returns the T_K tile for seed_kind 'x' (to seed the second level), else None.
    """
    sl = lambda r: UU[:, :, ubase + r]
    q = tmp_pool.tile([128, C], FP32, tag="lad_q")
    u = tmp_pool.tile([128, C], FP32, tag="lad_u")
    m = tmp_pool.tile([128, C], FP32, tag="lad_m")
    if seed_kind == "x":
        # T1 = ids/111 - 1 ; 2x^2 via ACT Square(sqrt2/111 * ids - sqrt2)
        nc.vector.tensor_scalar(sl(1), ids, 1.0 / 111.0, 1.0, ALU.mult, ALU.subtract)
        nc.scalar.activation(q[:], ids, AFT.Square, scale=R2 / 111.0, bias=bias_r2)
        s2 = tmp_pool.tile([128, C], FP32, tag="lad_s2")
        nc.vector.tensor_scalar(s2[:], ids, 2.0 / 111.0, 2.0, ALU.mult, ALU.subtract)
    else:
        nc.scalar.copy(sl(1), y)
        nc.scalar.activation(q[:], y, AFT.Square, scale=R2, bias=0.0)
        s2 = tmp_pool.tile([128, C], FP32, tag="lad_s2")
        nc.vector.tensor_scalar_mul(s2[:], y, 2.0)
    nc.gpsimd.memset(sl(0), 1.0)
    nc.vector.tensor_scalar_sub(sl(2), q[:], 1.0)          # T2 = 2x^2-1
    nc.vector.tensor_scalar(u[:], q[:], 2.0, 2.0, ALU.mult, ALU.subtract)  # u=2*T2
    # T3 = 2x*T2 - x
    nc.vector.tensor_tensor(m[:], s2[:], sl(2), ALU.mult)
    nc.vector.scalar_tensor_tensor(sl(3), m[:], 1.0, sl(1), ALU.mult, ALU.subtract)
    # T4 (shifted: slot = 2*T2^2 = T4+1)
    nc.scalar.activation(sl(4), sl(2), AFT.Square, scale=R2, bias=0.0)
    # odd chain on DVE: T_r = u*T_{r-2} - T_{r-4}
    for r in range(5, nb, 2):
        nc.vector.tensor_tensor(m[:], u[:], sl(r - 2), ALU.mult)
        nc.vector.scalar_tensor_tensor(sl(r), m[:], 1.0, sl(r - 4), ALU.mult,
                                       ALU.subtract)
    # evens >= 6: shifted squares (inputs with even m>=4 are shifted -> bias)
    for r in range(6, nb, 2):
        m2 = r // 2
        b = bias_r2 if (m2 >= 4 and m2 % 2 == 0) else 0.0
        nc.scalar.activation(sl(r), sl(m2), AFT.Square, scale=R2, bias=b)
    if seed_kind == "x":
        # y = T_21 = u*T_19 - T_17
        yt = tmp_pool.tile([128, C], FP32, tag="lad_y")
        nc.vector.tensor_tensor(m[:], u[:], sl(K - 2), ALU.mult)
        nc.vector.scalar_tensor_tensor(yt[:], m[:], 1.0, sl(K - 4), ALU.mult,
                                       ALU.subtract)
        return yt
    return None


def _build(slices=NSL):
    nc = bacc.Bacc("TRN2", target_bir_lowering=False, debug=False, num_devices=1)
    yp = nc.dram_tensor("yp", [BC, J, 2], FP32, kind="ExternalInput")
    yt = nc.dram_tensor("yt", [BC, J, 2], FP32, kind="ExternalInput")
    # signed/scaled G layout [44, SL*43]: rows 0..21 (+G/700) hit the T-eval
    # block (cols b*43+r, r<21); rows 22..43 (-G/700) hit P-block (cols 21+r).
    gc = nc.dram_tensor("gc", [2 * A, SL * NB], FP32, kind="ExternalInput")
    out3 = nc.dram_tensor("out3", [1, 4], FP32, kind="ExternalOutput")

    bias_np = np.broadcast_to(np.array([-np.sqrt(2.0), -222.0], np.float32),
                              (128, 2)).copy()
    bias_dram = nc.inline_tensor(bias_np, name="bias_const")
    yp_r = yp.ap().rearrange("b (p c) t -> p b c t", p=128)
    yt_r = yt.ap().rearrange("b (p c) t -> p b c t", p=128)

    with tile.TileContext(nc) as tc:
        with (
            tc.tile_pool(name="data", bufs=2) as data_pool,
            tc.tile_pool(name="basis", bufs=1) as basis_pool,
            tc.tile_pool(name="tmp", bufs=1) as tmp_pool,
            tc.tile_pool(name="small", bufs=1) as small_pool,
            tc.tile_pool(name="psum", bufs=2, space="PSUM") as psum_pool,
            tc.tile_pool(name="psc", bufs=2, space="PSUM") as psc_pool,
        ):
            gct = small_pool.tile([2 * A, SL * NB], FP32)
            nc.sync.dma_start(gct[:], gc.ap())
            bias_t = small_pool.tile([128, 2], FP32)
            nc.sync.dma_start(bias_t[:], bias_dram.ap())
            ones44 = small_pool.tile([2 * A, 1], FP32)
            nc.gpsimd.memset(ones44[:], 1.0)
            ones128 = small_pool.tile([128, 1], FP32)
            nc.gpsimd.memset(ones128[:], 1.0)
            diffs = small_pool.tile([1, BC], FP32)
            pen_parts = small_pool.tile([128, NSL], FP32)
            ir_parts = small_pool.tile([128, NSL], FP32)

            for s in range(slices):
                cs = slice(s * C, (s + 1) * C)
                dP = data_pool.tile([128, C, 2], FP32, tag="dP")
                dT = data_pool.tile([128, C, 2], FP32, tag="dT")
                bs = slice(s * SL, (s + 1) * SL)
                nc.sync.dma_start(
                    dP[:].rearrange("p (b c) t -> p b c t", b=SL), yp_r[:, bs, :, :])
                nc.sync.dma_start(
                    dT[:].rearrange("p (b c) t -> p b c t", b=SL), yt_r[:, bs, :, :])
                idsP, amtP = dP[:, :, 0], dP[:, :, 1]
                idsT, amtT = dT[:, :, 0], dT[:, :, 1]

                # round pred ids (cast rte matches jnp.round)
                ki = tmp_pool.tile([128, C], mybir.dt.int32, tag="lad_q")
                kf = tmp_pool.tile([128, C], FP32, tag="kf")
                nc.vector.tensor_copy(ki[:], idsP)
                nc.vector.tensor_copy(kf[:], ki[:])

                # penalties on pred slice (raw ids/amt)
                t_i = tmp_pool.tile([128, C], FP32, tag="lad_u")
                t_a = tmp_pool.tile([128, C], FP32, tag="lad_m")
                pm = tmp_pool.tile([128, C], FP32, tag="lad_s2")
                nc.scalar.activation(t_i[:], idsP, AFT.Tanh, scale=4.0)
                nc.scalar.activation(t_a[:], amtP, AFT.Tanh, scale=4.0)
                nc.gpsimd.tensor_tensor(pm[:], t_i[:], t_a[:], ALU.mult)
                nc.vector.scalar_tensor_tensor(pm[:], pm[:], -2.0, t_i[:],
                                               ALU.mult, ALU.add)
                nc.vector.scalar_tensor_tensor(
                    pm[:], pm[:], 1.0, t_a[:], ALU.mult, ALU.add,
                    accum_out=pen_parts[:, s:s + 1])
                ir_t = tmp_pool.tile([128, C], FP32, tag="lad_y")
                nc.scalar.activation(ir_t[:], idsP, AFT.Relu, bias=bias_t[:, 1:2],
                                     accum_out=ir_parts[:, s:s + 1])

                # basis tiles: UU [128, 43, C] (U_T 0..20 | U_P 21..41+1),
                # WW [128, 44, C] (amt*V: T rows 0..21 | P rows 22..43)
                UU = basis_pool.tile([128, C, NB], FP32, tag="UU")
                WW = basis_pool.tile([128, C, 2 * A], FP32, tag="WW")

                for (ids_ap, amt_ap, ub, wb) in (
                    (idsT, amtT, 0, 0),
                    (kf[:], amtP, K, A),
                ):
                    y_t = _build_ladder(nc, bias_t[:, 0:1], tmp_pool, UU, ub, K,
                                        "x", ids=ids_ap)
                    _build_ladder(nc, bias_t[:, 0:1], tmp_pool, WW, wb, A, "y", y=y_t[:])
                    # fold amt into V rows in place (split DVE / gpsimd)
                    for a in range(A):
                        eng = nc.gpsimd if a < 16 else nc.vector
                        eng.tensor_tensor(WW[:, :, wb + a], WW[:, :, wb + a], amt_ap,
                                          ALU.mult)

                # per-batch Grams: psum [44, SL*43], accumulate over 56 chunks
                ps = psum_pool.tile([2 * A, SL * NB], FP32, tag="gram")
                for b in range(SL):
                    for c in range(CHUNKS):
                        j = b * CHUNKS + c
                        nc.tensor.matmul(
                            ps[:, b * NB:(b + 1) * NB],
                            WW[:, j, :], UU[:, j, :],
                            start=(c == 0), stop=(c == CHUNKS - 1))
                # contract with signed G: diffs[b] = calT - calP
                gs = tmp_pool.tile([2 * A, SL * NB], FP32, tag="gs")
                nc.vector.scalar_tensor_tensor(gs[:], ps[:], 1.0, gct[:],
                                               ALU.mult, ALU.mult)
                ps2 = psc_pool.tile([1, SL * NB], FP32, tag="colsum")
                nc.tensor.matmul(ps2[:], ones44[:], gs[:], start=True, stop=True)
                sall = tmp_pool.tile([1, SL * NB], FP32, tag="sall")
                nc.scalar.copy(sall[:], ps2[:])
                nc.vector.tensor_reduce(
                    diffs[:, s * SL:(s + 1) * SL],
                    sall[:].rearrange("p (b n) -> p b n", n=NB),
                    mybir.AxisListType.X, ALU.add)

            # final: sum_b diffs^2, penalty partition-sums
            dsq = small_pool.tile([1, BC], FP32)
            nc.scalar.activation(dsq[:], diffs[:], AFT.Square)
            v0 = small_pool.tile([1, 1], FP32)
            nc.vector.tensor_reduce(v0[:], dsq[:], mybir.AxisListType.X, ALU.add)
            pen_red = small_pool.tile([128, 2], FP32)
            nc.vector.tensor_reduce(pen_red[:, 0:1], pen_parts[:],
                                    mybir.AxisListType.X, ALU.add)
            nc.vector.tensor_reduce(pen_red[:, 1:2], ir_parts[:],
                                    mybir.AxisListType.X, ALU.add)
            ps3 = psc_pool.tile([1, 2], FP32, tag="pen")
            nc.tensor.matmul(ps3[:], ones128[:], pen_red[:], start=True, stop=True)
            ot = small_pool.tile([1, 4], FP32)
            nc.vector.tensor_copy(ot[:, 0:1], v0[:])
            nc.vector.tensor_copy(ot[:, 1:3], ps3[:])
            nc.gpsimd.memset(ot[:, 3:4], 0.0)
            nc.sync.dma_start(out3.ap(), ot[:])
    nc.compile()
    return nc


@functools.lru_cache(maxsize=2)
def _compiled():
    return _build()


def kernel(y_pred: np.ndarray, y: np.ndarray, calories_coeffs: np.ndarray,
           _trace: bool = False):
    G = _fold_G(np.asarray(calories_coeffs, np.float64))
    gc = np.zeros((2 * A, SL * NB), np.float32)
    for b in range(SL):
        gc[:A, b * NB:b * NB + K] = (G / 700.0).astype(np.float32)
        gc[A:, b * NB + K:b * NB + 2 * K] = (-G / 700.0).astype(np.float32)

    ypf = np.ascontiguousarray(y_pred.reshape(B, J, 2), np.float32)
    ytf = np.ascontiguousarray(y.reshape(B, J, 2), np.float32)
    in_maps = []
    for i in range(N_CORES):
        in_maps.append({
            "yp": ypf[i * BC:(i + 1) * BC],
            "yt": ytf[i * BC:(i + 1) * BC],
            "gc": gc,
        })
    nc = _compiled()
    res = run_bass_kernel_spmd(nc, in_maps, list(range(N_CORES)), trace=_trace)
    parts = np.stack([r["out3"][0] for r in res.results])  # [8, 4]
    tot = parts.sum(axis=0)
    loss = (tot[0] + tot[1] + tot[2]) / float(B)
    out = np.float32(loss)
    if _trace:
        return out, res
    return out



# revision 6
# speedup vs baseline: 3.1816x; 3.0146x over previous
"""MenuLoss Trainium2 kernel (v4).

Math: per batch b, cal[b] = (1/700)*sum_j amt_bj * p(x_bj) for two evals (true
ids continuous f32, pred ids rounded to integers), p a deg-446 Chebyshev series.
Factor p(x) = sum_{a<28, r<16} G[a,r] * w_a(x) * t_r(x) where the device basis
columns w_a (deg 16a) and t_r (deg r) are built by a fixed recipe: an f32
SHIFTED backbone chain sh_m = T_m + 1 via ACT Square((sqrt2)z - sqrt2) = 2(z-1)^2
(exact Chebyshev doubling, immune to f16 depth amplification), unshifted to f16
single-rounding leaf columns, plus grouped f16 DVE products for the rest.  G is
solved on host in f64 against the exact recipe polynomials (cond ~350).  amt is
folded into the t-side (f16), per-batch Grams accumulate in PSUM via fp16
TensorE matmuls (1 cyc/row vs 4 for fp32), and a signed G contraction yields
calT[b]-calP[b] directly.  Penalties ride along via ACT-accumulated tanh/relu
sums.  8-way batch data parallel, per-core scalars combined on host.
"""
import functools
import sys
import types
import numpy as np
import numpy.polynomial.chebyshev as Ch

if "antenv.axon_hooks" not in sys.modules:
    _m = types.ModuleType("antenv.axon_hooks")
    _m.get_axon_ntff_profile_hook = lambda: None
    sys.modules["antenv.axon_hooks"] = _m

import concourse.bacc as bacc
import concourse.bass as bass
import concourse.mybir as mybir
import concourse.tile as tile
from concourse.bass_utils import run_bass_kernel_spmd

AFT = mybir.ActivationFunctionType
ALU = mybir.AluOpType
F32 = mybir.dt.float32
F16 = mybir.dt.float16
I32 = mybir.dt.int32

N_CORES = 8
B, J = 512, 7 * 16 * 64          # 512 batches, 7168 elements/batch
BC = B // N_CORES                # 64 batches per core
SL = 8                           # batches per slice
NSL = BC // SL                   # 8 slices
CH = J // 128                    # 56 chunk columns per batch
C = SL * CH                      # 448 columns per slice
A, R = 28, 16                    # p = sum G[a,r] w_a t_r ; deg = 16a + r
WR = 1 + 27 + 27                 # W rows: 0=ones | 1..27 true w_a | 28..54 pred
UR = 2 * R                       # U rows: 0..15 true amt*t_r | 16..31 pred
R2C = float(np.sqrt(2.0))


# ---------------- host-side basis recipe mirror + G solve ----------------
def _xladder_ops():
    # scratch idx k = t_{k+1}; idx0 = seed t_1
    return [("dbl1", 1, 0), ("mul", 2, 1, 0), ("dbl1", 3, 1), ("mulg", 4, 3, 0, 3),
            ("dbl1", 7, 3), ("mulg", 8, 7, 0, 7)]


def _wladder_ops():
    # idx a = w_a; idx1 = psi = T_16 exact
    return [("dbl1", 2, 1), ("mul", 3, 2, 1), ("dbl1", 4, 2), ("mulg", 5, 4, 1, 3),
            ("dbl1", 8, 4), ("mulg", 9, 8, 1, 7), ("dbl1", 16, 8),
            ("mulg", 17, 16, 1, 11)]


def _mir_dbl1(z):
    p = 2.0 * Ch.chebmul(z, z)
    p[0] -= 1.0
    return p


def _run_poly(cols, ops):
    for op in ops:
        if op[0] == "dbl1":
            cols[op[1]] = _mir_dbl1(cols[op[2]])
        elif op[0] == "mul":
            cols[op[1]] = Ch.chebmul(cols[op[2]], cols[op[3]])
        else:
            _, dst, srcb, src0, w = op
            for k in range(w):
                cols[dst + k] = Ch.chebmul(cols[srcb], cols[src0 + k])
    return cols


def _solve_G(coeffs447: np.ndarray) -> np.ndarray:
    xc = {0: np.array([1.0])}
    scr = _run_poly({0: np.array([0.0, 1.0])}, _xladder_ops())
    for k in range(15):
        xc[k + 1] = scr[k]
    w = _run_poly({1: _mir_dbl1(scr[7])}, _wladder_ops())
    w[0] = np.array([1.0])
    M = np.zeros((448, 448))
    for a in range(A):
        for r in range(R):
            pr = Ch.chebmul(w[a], xc[r])
            M[: len(pr), a * R + r] = pr
    c = np.zeros(448)
    c[:447] = coeffs447
    return np.linalg.solve(M, c).reshape(A, R)


# ---------------- device kernel ----------------
def _ladder(nc, X, BB, WW, UU, seed_ap, seed_f32, wbase, ubase, bias_r2):
    """Build one side: scratch t-cols in X[0..14], backbone in BB (f32,
    shifted), W rows WW[wbase+a] (a=1..27), U rows UU[ubase+r] (r=1..15,
    r=0 = amt pre-DMA'd).  seed_ap: ids tile; seed_f32: True if f32 input."""
    k = 1.0 / 111.0
    # backbone seed sh_1 = x~ + 1 = ids/111 (f32) ; f16 leaf t_1 = ids/111 - 1
    nc.vector.tensor_scalar(BB[:, 0, :], seed_ap, k, 0.0, ALU.mult, ALU.add)
    nc.vector.tensor_scalar(X[:, 0, :], seed_ap, k, 1.0, ALU.mult, ALU.subtract)
    # f32 shifted chain on ACT: sh_2, sh_4, sh_8 then psi=sh_16, w-backbone
    for i in range(8):
        nc.scalar.activation(BB[:, i + 1, :], BB[:, i, :], AFT.Square,
                             scale=R2C, bias=bias_r2)
    # unshift to f16: t2, t4, t8 ; psi -> W row wbase+1 ; w2,w4 ; w8,w16
    for (src, dst) in ((1, X[:, 1, :]), (2, X[:, 3, :]), (3, X[:, 7, :]),
                      (4, WW[:, wbase + 1, :])):
        nc.vector.tensor_scalar(dst, BB[:, src, :], 1.0, 1.0, ALU.mult,
                                ALU.subtract)
    nc.vector.tensor_scalar(WW[:, wbase + 2:wbase + 5:2, :], BB[:, 5:7, :],
                            1.0, 1.0, ALU.mult, ALU.subtract)
    nc.vector.tensor_scalar(WW[:, wbase + 8:wbase + 17:8, :], BB[:, 7:9, :],
                            1.0, 1.0, ALU.mult, ALU.subtract)
    # f16 leaf products (grouped, broadcast first operand)
    def mulg(eng, dst_ap, bc_ap, in_ap, w):
        bc = bc_ap.unsqueeze(1).broadcast_to((128, w, C))
        eng.tensor_tensor(dst_ap, bc, in_ap, ALU.mult)
    v, g = nc.vector, nc.gpsimd
    # scratch: t3 ; t5..7 ; t9..15
    v.tensor_tensor(X[:, 2, :], X[:, 1, :], X[:, 0, :], ALU.mult)
    mulg(v, X[:, 4:7, :], X[:, 3, :], X[:, 0:3, :], 3)
    mulg(v, X[:, 8:15, :], X[:, 7, :], X[:, 0:7, :], 7)
    # folds: U rows 1..15 = amt * t_r
    mulg(v, UU[:, ubase + 1:ubase + 16, :], UU[:, ubase, :], X[:, 0:15, :], 15)
    # W leaves: w3 ; w5..7 ; w9..15 ; w17..27
    wb = wbase
    v.tensor_tensor(WW[:, wb + 3, :], WW[:, wb + 2, :], WW[:, wb + 1, :], ALU.mult)
    mulg(g, WW[:, wb + 5:wb + 8, :], WW[:, wb + 4, :], WW[:, wb + 1:wb + 4, :], 3)
    mulg(v, WW[:, wb + 9:wb + 16, :], WW[:, wb + 8, :], WW[:, wb + 1:wb + 8, :], 7)
    mulg(v, WW[:, wb + 17:wb + 28, :], WW[:, wb + 16, :], WW[:, wb + 1:wb + 12, :], 11)


def _build(slices=NSL):
    nc = bacc.Bacc("TRN2", target_bir_lowering=False, debug=False, num_devices=1)
    ip = nc.dram_tensor("ip", [BC, J], F32, kind="ExternalInput")   # pred ids raw
    pa = nc.dram_tensor("pa", [BC, J], F16, kind="ExternalInput")   # pred amt
    it = nc.dram_tensor("it", [BC, J], F32, kind="ExternalInput")   # true ids
    ta = nc.dram_tensor("ta", [BC, J], F16, kind="ExternalInput")   # true amt
    gc = nc.dram_tensor("gc", [WR, SL * UR], F32, kind="ExternalInput")
    out = nc.dram_tensor("out", [1, 8], F32, kind="ExternalOutput")

    bias_np = np.broadcast_to(
        np.array([-np.sqrt(2.0), -222.0], np.float32), (128, 2)).copy()
    bias_dram = nc.inline_tensor(bias_np, name="bias_const")

    ip_r = ip.ap().rearrange("b (p c) -> p b c", p=128)
    pa_r = pa.ap().rearrange("b (p c) -> p b c", p=128)
    it_r = it.ap().rearrange("b (p c) -> p b c", p=128)
    ta_r = ta.ap().rearrange("b (p c) -> p b c", p=128)

    with tile.TileContext(nc) as tc:
        with (
            tc.tile_pool(name="data", bufs=2) as data_pool,
            tc.tile_pool(name="basis", bufs=2) as basis_pool,
            tc.tile_pool(name="scr", bufs=1) as scr_pool,
            tc.tile_pool(name="small", bufs=1) as small_pool,
            tc.tile_pool(name="psum", bufs=2, space="PSUM") as psum_pool,
            tc.tile_pool(name="psc", bufs=2, space="PSUM") as psc_pool,
        ):
            gct = small_pool.tile([WR, SL * UR], F32)
            nc.sync.dma_start(gct[:], gc.ap())
            bias_t = small_pool.tile([128, 2], F32)
            nc.sync.dma_start(bias_t[:], bias_dram.ap())
            ones55 = small_pool.tile([WR, 1], F16)
            nc.gpsimd.memset(ones55[:], 1.0)
            ones128 = small_pool.tile([128, 1], F16)
            nc.gpsimd.memset(ones128[:], 1.0)
            diffs = small_pool.tile([1, BC], F32)
            pen_i = small_pool.tile([128, NSL], F32)
            pen_a = small_pool.tile([128, NSL], F32)
            pen_m = small_pool.tile([128, NSL], F32)
            pen_r = small_pool.tile([128, NSL], F32)

            for s in range(slices):
                bs = slice(s * SL, (s + 1) * SL)
                PI = data_pool.tile([128, C], F32, tag="PI")
                TI = data_pool.tile([128, C], F32, tag="TI")
                WW = basis_pool.tile([128, WR, C], F16, tag="WW")
                UU = basis_pool.tile([128, UR, C], F16, tag="UU")
                r3 = lambda ap_: ap_.rearrange("p (b c) -> p b c", b=SL)
                nc.sync.dma_start(r3(PI[:]), ip_r[:, bs, :])
                nc.sync.dma_start(r3(TI[:]), it_r[:, bs, :])
                nc.sync.dma_start(r3(UU[:, 0, :]), ta_r[:, bs, :])
                nc.sync.dma_start(r3(UU[:, R, :]), pa_r[:, bs, :])

                X = scr_pool.tile([128, 15, C], F16, tag="X")
                BB = scr_pool.tile([128, 9, C], F32, tag="BB")
                nc.gpsimd.memset(WW[:, 0, :], 1.0)

                # penalties on raw pred (tanh/relu accumulate on ACT)
                ti = scr_pool.tile([128, C], F16, tag="ti")
                ta_t = scr_pool.tile([128, C], F16, tag="ta")
                tm = scr_pool.tile([128, C], F16, tag="tm")
                nc.scalar.activation(ti[:], PI[:], AFT.Tanh, scale=4.0,
                                     accum_out=pen_i[:, s:s + 1])
                nc.scalar.activation(ta_t[:], UU[:, R, :], AFT.Tanh, scale=4.0,
                                     accum_out=pen_a[:, s:s + 1])
                nc.vector.scalar_tensor_tensor(tm[:], ti[:], 1.0, ta_t[:],
                                               ALU.mult, ALU.mult,
                                               accum_out=pen_m[:, s:s + 1])
                rl = scr_pool.tile([128, C], F16, tag="rl")
                nc.scalar.activation(rl[:], PI[:], AFT.Relu, bias=bias_t[:, 1:2],
                                     accum_out=pen_r[:, s:s + 1])

                # round pred ids (rte int convert, on Pool)
                ki = scr_pool.tile([128, C], I32, tag="ki")
                kf = scr_pool.tile([128, C], F16, tag="kf")
                nc.gpsimd.tensor_copy(ki[:], PI[:])
                nc.gpsimd.tensor_copy(kf[:], ki[:])

                _ladder(nc, X, BB, WW, UU, TI[:], True, 0, 0, bias_t[:, 0:1])
                _ladder(nc, X, BB, WW, UU, kf[:], False, 27, R, bias_t[:, 0:1])

                # per-batch Grams: accumulate CH chunks into PSUM
                ps = psum_pool.tile([WR, SL * UR], F32, tag="gram")
                for b in range(SL):
                    for cc in range(CH):
                        j = b * CH + cc
                        nc.tensor.matmul(ps[:, b * UR:(b + 1) * UR],
                                         WW[:, :, j], UU[:, :, j],
                                         start=(cc == 0), stop=(cc == CH - 1))
                gs = scr_pool.tile([WR, SL * UR], F16, tag="gs")
                nc.vector.scalar_tensor_tensor(gs[:], ps[:], 1.0, gct[:],
                                               ALU.mult, ALU.mult)
                ps2 = psc_pool.tile([1, SL * UR], F32, tag="colsum")
                nc.tensor.matmul(ps2[:], ones55[:], gs[:], start=True, stop=True)
                sall = scr_pool.tile([1, SL * UR], F32, tag="sall")
                nc.scalar.copy(sall[:], ps2[:])
                nc.vector.tensor_reduce(
                    diffs[:, s * SL:(s + 1) * SL],
                    sall[:].rearrange("p (b n) -> p b n", n=UR),
                    mybir.AxisListType.X, ALU.add)

            # final: v0 = sum_b diffs^2 ; penalty partition sums
            dsq = small_pool.tile([1, BC], F32)
            nc.scalar.activation(dsq[:], diffs[:], AFT.Square)
            v0 = small_pool.tile([1, 1], F32)
            nc.vector.tensor_reduce(v0[:], dsq[:], mybir.AxisListType.X, ALU.add)
            pen_red = small_pool.tile([128, 4], F16)
            with nc.allow_low_precision(reason="penalty sums are O(10) scalars"):
                for idx, t in enumerate((pen_i, pen_a, pen_m, pen_r)):
                    nc.vector.tensor_reduce(pen_red[:, idx:idx + 1], t[:],
                                            mybir.AxisListType.X, ALU.add)
            ps3 = psc_pool.tile([1, 4], F32, tag="pen")
            nc.tensor.matmul(ps3[:], ones128[:], pen_red[:], start=True, stop=True)
            ot = small_pool.tile([1, 8], F32)
            nc.vector.tensor_copy(ot[:, 0:1], v0[:])
            nc.vector.tensor_copy(ot[:, 1:5], ps3[:])
            nc.gpsimd.memset(ot[:, 5:8], 0.0)
            nc.sync.dma_start(out.ap(), ot[:])
    nc.compile()
    return nc


@functools.lru_cache(maxsize=2)
def _compiled():
    return _build()


def kernel(y_pred: np.ndarray, y: np.ndarray, calories_coeffs: np.ndarray,
           _trace: bool = False):
    G = _solve_G(np.asarray(calories_coeffs, np.float64)) / 700.0
    gcv = np.zeros((WR, SL * UR), np.float32)
    for b in range(SL):
        blk = gcv[:, b * UR:(b + 1) * UR]
        blk[0, 0:R] = G[0]
        blk[1:A, 0:R] = G[1:]
        blk[0, R:2 * R] = -G[0]
        blk[A:WR, R:2 * R] = -G[1:]

    yp = np.asarray(y_pred, np.float32).reshape(B, J, 2)
    yt = np.asarray(y, np.float32).reshape(B, J, 2)
    ip_h = np.ascontiguousarray(yp[:, :, 0])
    pa_h = np.ascontiguousarray(yp[:, :, 1].astype(np.float16))
    it_h = np.ascontiguousarray(yt[:, :, 0])
    ta_h = np.ascontiguousarray(yt[:, :, 1].astype(np.float16))
    in_maps = []
    for i in range(N_CORES):
        sl_ = slice(i * BC, (i + 1) * BC)
        in_maps.append({"ip": ip_h[sl_], "pa": pa_h[sl_], "it": it_h[sl_],
                        "ta": ta_h[sl_], "gc": gcv})
    nc = _compiled()
    res = run_bass_kernel_spmd(nc, in_maps, list(range(N_CORES)), trace=_trace)
    parts = np.stack([r["out"][0] for r in res.results])  # [8, 8]
    tot = parts.sum(axis=0).astype(np.float64)
    v0, a1, a2, a3, rl = tot[0], tot[1], tot[2], tot[3], tot[4]
    loss = (v0 + (a1 + a2 - 2.0 * a3) + rl) / float(B)
    outv = np.float32(loss)
    if _trace:
        return outv, res
    return outv


# revision 9
# speedup vs baseline: 3.7059x; 1.1648x over previous
"""MenuLoss Trainium2 kernel (v4).

Math: per batch b, cal[b] = (1/700)*sum_j amt_bj * p(x_bj) for two evals (true
ids continuous f32, pred ids rounded to integers), p a deg-446 Chebyshev series.
Factor p(x) = sum_{a<28, r<16} G[a,r] * w_a(x) * t_r(x) where the device basis
columns w_a (deg 16a) and t_r (deg r) are built by a fixed recipe: an f32
SHIFTED backbone chain sh_m = T_m + 1 via ACT Square((sqrt2)z - sqrt2) = 2(z-1)^2
(exact Chebyshev doubling, immune to f16 depth amplification), unshifted to f16
single-rounding leaf columns, plus grouped f16 DVE products for the rest.  G is
solved on host in f64 against the exact recipe polynomials (cond ~350).  amt is
folded into the t-side (f16), per-batch Grams accumulate in PSUM via fp16
TensorE matmuls (1 cyc/row vs 4 for fp32), and a signed G contraction yields
calT[b]-calP[b] directly.  Penalties ride along via ACT-accumulated tanh/relu
sums.  8-way batch data parallel, per-core scalars combined on host.
"""
import functools
import sys
import types
import numpy as np
import numpy.polynomial.chebyshev as Ch

if "antenv.axon_hooks" not in sys.modules:
    _m = types.ModuleType("antenv.axon_hooks")
    _m.get_axon_ntff_profile_hook = lambda: None
    sys.modules["antenv.axon_hooks"] = _m

import concourse.bacc as bacc
import concourse.bass as bass
import concourse.mybir as mybir
import concourse.tile as tile
from concourse.bass_utils import run_bass_kernel_spmd

AFT = mybir.ActivationFunctionType
ALU = mybir.AluOpType
F32 = mybir.dt.float32
F16 = mybir.dt.float16
I32 = mybir.dt.int32

N_CORES = 8
B, J = 512, 7 * 16 * 64          # 512 batches, 7168 elements/batch
BC = B // N_CORES                # 64 batches per core
SL = 8                           # batches per slice
NSL = BC // SL                   # 8 slices
CH = J // 128                    # 56 chunk columns per batch
C = SL * CH                      # 448 columns per slice
A, R = 28, 16                    # p = sum G[a,r] w_a t_r ; deg = 16a + r
WR = 1 + 27 + 27                 # W rows: 0=ones | 1..27 true w_a | 28..54 pred
UR = 2 * R                       # U rows: 0..15 true amt*t_r | 16..31 pred
R2C = float(np.sqrt(2.0))


# ---------------- host-side basis recipe mirror + G solve ----------------
def _xladder_ops():
    # scratch idx k = t_{k+1}; idx0 = seed t_1
    return [("dbl1", 1, 0), ("mul", 2, 1, 0), ("dbl1", 3, 1), ("mulg", 4, 3, 0, 3),
            ("dbl1", 7, 3), ("mulg", 8, 7, 0, 7)]


def _wladder_ops():
    # idx a = w_a; idx1 = psi = T_16 exact
    return [("dbl1", 2, 1), ("mul", 3, 2, 1), ("dbl1", 4, 2), ("mulg", 5, 4, 1, 3),
            ("dbl1", 8, 4), ("mulg", 9, 8, 1, 7), ("dbl1", 16, 8),
            ("mulg", 17, 16, 1, 11)]


def _mir_dbl1(z):
    p = 2.0 * Ch.chebmul(z, z)
    p[0] -= 1.0
    return p


def _run_poly(cols, ops):
    for op in ops:
        if op[0] == "dbl1":
            cols[op[1]] = _mir_dbl1(cols[op[2]])
        elif op[0] == "mul":
            cols[op[1]] = Ch.chebmul(cols[op[2]], cols[op[3]])
        else:
            _, dst, srcb, src0, w = op
            for k in range(w):
                cols[dst + k] = Ch.chebmul(cols[srcb], cols[src0 + k])
    return cols


def _solve_G(coeffs447: np.ndarray) -> np.ndarray:
    xc = {0: np.array([1.0])}
    scr = _run_poly({0: np.array([0.0, 1.0])}, _xladder_ops())
    for k in range(15):
        xc[k + 1] = scr[k]
    w = _run_poly({1: _mir_dbl1(scr[7])}, _wladder_ops())
    w[0] = np.array([1.0])
    M = np.zeros((448, 448))
    for a in range(A):
        for r in range(R):
            pr = Ch.chebmul(w[a], xc[r])
            M[: len(pr), a * R + r] = pr
    c = np.zeros(448)
    c[:447] = coeffs447
    return np.linalg.solve(M, c).reshape(A, R)


# ---------------- device kernel ----------------
def _ladder(nc, X, BB, WW, UU, seed_ap, seed_f32, wbase, ubase, bias_r2,
            bias_m1):
    """Build one side: scratch t-cols in X[0..7] (= t_1..t_8), backbone in BB
    (f32, shifted), W rows WW[wbase+a] (a=1..27), U rows UU[ubase+r]
    (r=1..15, r=0 = amt pre-DMA'd).  U leaves reuse folded backbone cols:
    u_{4+k} = u4*t_k, u_{8+k} = u8*t_k."""
    k = 1.0 / 111.0
    # backbone seed sh_1 = x~ + 1 = ids/111 (f32) ; f16 leaf t_1 = ids/111 - 1
    nc.vector.tensor_scalar(BB[:, 0, :], seed_ap, k, 0.0, ALU.mult, ALU.add)
    nc.vector.tensor_scalar(X[:, 0, :], seed_ap, k, 1.0, ALU.mult, ALU.subtract)
    # f32 shifted chain on ACT: sh_2, sh_4, sh_8 then psi=sh_16, w-backbone
    for i in range(8):
        nc.scalar.activation(BB[:, i + 1, :], BB[:, i, :], AFT.Square,
                             scale=R2C, bias=bias_r2)
    # unshift to f16 on ACT (Copy with bias -1): t2, t4, t8 ; psi -> W row 1
    for (src, dst) in ((1, X[:, 1, :]), (2, X[:, 3, :]), (3, X[:, 7, :]),
                      (4, WW[:, wbase + 1, :])):
        nc.scalar.activation(dst, BB[:, src, :], AFT.Copy, bias=-1.0)
    nc.vector.tensor_scalar(WW[:, wbase + 2:wbase + 5:2, :], BB[:, 5:7, :],
                            1.0, 1.0, ALU.mult, ALU.subtract)
    nc.vector.tensor_scalar(WW[:, wbase + 8:wbase + 17:8, :], BB[:, 7:9, :],
                            1.0, 1.0, ALU.mult, ALU.subtract)
    # f16 leaf products (grouped, broadcast first operand)
    def mulg(eng, dst_ap, bc_ap, in_ap, w):
        bc = bc_ap.unsqueeze(1).broadcast_to((128, w, C))
        eng.tensor_tensor(dst_ap, bc, in_ap, ALU.mult)
    v, g = nc.vector, nc.gpsimd
    ub, wb = ubase, wbase
    # scratch leaves: t3 ; t5..7
    v.tensor_tensor(X[:, 2, :], X[:, 1, :], X[:, 0, :], ALU.mult)
    mulg(g, X[:, 4:7, :], X[:, 3, :], X[:, 0:3, :], 3)
    # U side: fold amt into backbone cols, derive the rest from folded cols
    mulg(v, UU[:, ub + 1:ub + 3, :], UU[:, ub, :], X[:, 0:2, :], 2)  # u1,u2
    v.tensor_tensor(UU[:, ub + 4, :], UU[:, ub, :], X[:, 3, :], ALU.mult)
    v.tensor_tensor(UU[:, ub + 8, :], UU[:, ub, :], X[:, 7, :], ALU.mult)
    v.tensor_tensor(UU[:, ub + 3, :], UU[:, ub + 2, :], X[:, 0, :], ALU.mult)
    mulg(g, UU[:, ub + 5:ub + 8, :], UU[:, ub + 4, :], X[:, 0:3, :], 3)
    mulg(v, UU[:, ub + 9:ub + 16, :], UU[:, ub + 8, :], X[:, 0:7, :], 7)
    # W leaves: w3 ; w5..7 ; w9..15 ; w17..27
    v.tensor_tensor(WW[:, wb + 3, :], WW[:, wb + 2, :], WW[:, wb + 1, :], ALU.mult)
    mulg(g, WW[:, wb + 5:wb + 8, :], WW[:, wb + 4, :], WW[:, wb + 1:wb + 4, :], 3)
    mulg(v, WW[:, wb + 9:wb + 16, :], WW[:, wb + 8, :], WW[:, wb + 1:wb + 8, :], 7)
    mulg(v, WW[:, wb + 17:wb + 28, :], WW[:, wb + 16, :], WW[:, wb + 1:wb + 12, :], 11)


def _build(slices=NSL):
    nc = bacc.Bacc("TRN2", target_bir_lowering=False, debug=False, num_devices=1)
    ip = nc.dram_tensor("ip", [BC, J], F32, kind="ExternalInput")   # pred ids raw
    pa = nc.dram_tensor("pa", [BC, J], F16, kind="ExternalInput")   # pred amt
    it = nc.dram_tensor("it", [BC, J], F32, kind="ExternalInput")   # true ids
    ta = nc.dram_tensor("ta", [BC, J], F16, kind="ExternalInput")   # true amt
    gc = nc.dram_tensor("gc", [WR, SL * UR], F32, kind="ExternalInput")
    out = nc.dram_tensor("out", [1, 8], F32, kind="ExternalOutput")

    bias_np = np.broadcast_to(
        np.array([-np.sqrt(2.0), -222.0, -1.0], np.float32), (128, 3)).copy()
    bias_dram = nc.inline_tensor(bias_np, name="bias_const")

    ip_r = ip.ap().rearrange("b (p c) -> p b c", p=128)
    pa_r = pa.ap().rearrange("b (p c) -> p b c", p=128)
    it_r = it.ap().rearrange("b (p c) -> p b c", p=128)
    ta_r = ta.ap().rearrange("b (p c) -> p b c", p=128)

    with tile.TileContext(nc) as tc:
        with (
            tc.tile_pool(name="data", bufs=2) as data_pool,
            tc.tile_pool(name="basis", bufs=2) as basis_pool,
            tc.tile_pool(name="scr", bufs=1) as scr_pool,
            tc.tile_pool(name="small", bufs=1) as small_pool,
            tc.tile_pool(name="psum", bufs=2, space="PSUM") as psum_pool,
            tc.tile_pool(name="psc", bufs=2, space="PSUM") as psc_pool,
        ):
            gct = small_pool.tile([WR, SL * UR], F32)
            nc.sync.dma_start(gct[:], gc.ap())
            bias_t = small_pool.tile([128, 3], F32)
            nc.sync.dma_start(bias_t[:], bias_dram.ap())
            ones55 = small_pool.tile([WR, 1], F16)
            nc.gpsimd.memset(ones55[:], 1.0)
            ones128 = small_pool.tile([128, 1], F16)
            nc.gpsimd.memset(ones128[:], 1.0)
            diffs = small_pool.tile([1, BC], F32)
            pen_i = small_pool.tile([128, NSL], F32)
            pen_a = small_pool.tile([128, NSL], F32)
            pen_m = small_pool.tile([128, NSL], F32)
            pen_r = small_pool.tile([128, NSL], F32)

            for s in range(slices):
                bs = slice(s * SL, (s + 1) * SL)
                PI = data_pool.tile([128, C], F32, tag="PI")
                TI = data_pool.tile([128, C], F32, tag="TI")
                WW = basis_pool.tile([128, WR, C], F16, tag="WW")
                UU = basis_pool.tile([128, UR, C], F16, tag="UU")
                r3 = lambda ap_: ap_.rearrange("p (b c) -> p b c", b=SL)
                nc.sync.dma_start(r3(PI[:]), ip_r[:, bs, :])
                nc.sync.dma_start(r3(TI[:]), it_r[:, bs, :])
                nc.sync.dma_start(r3(UU[:, 0, :]), ta_r[:, bs, :])
                nc.sync.dma_start(r3(UU[:, R, :]), pa_r[:, bs, :])

                X = scr_pool.tile([128, 8, C], F16, tag="X")
                BB = scr_pool.tile([128, 9, C], F32, tag="BB")
                nc.gpsimd.memset(WW[:, 0, :], 1.0)

                # penalties on raw pred (tanh/relu accumulate on ACT)
                ti = scr_pool.tile([128, C], F16, tag="ti")
                ta_t = scr_pool.tile([128, C], F16, tag="ta")
                tm = scr_pool.tile([128, C], F16, tag="tm")
                nc.scalar.activation(ti[:], PI[:], AFT.Tanh, scale=4.0,
                                     accum_out=pen_i[:, s:s + 1])
                nc.scalar.activation(ta_t[:], UU[:, R, :], AFT.Tanh, scale=4.0,
                                     accum_out=pen_a[:, s:s + 1])
                nc.vector.scalar_tensor_tensor(tm[:], ti[:], 1.0, ta_t[:],
                                               ALU.mult, ALU.mult,
                                               accum_out=pen_m[:, s:s + 1])
                rl = scr_pool.tile([128, C], F16, tag="rl")
                nc.scalar.activation(rl[:], PI[:], AFT.Relu, bias=bias_t[:, 1:2],
                                     accum_out=pen_r[:, s:s + 1])

                # round pred ids (rte int convert, on Pool)
                ki = scr_pool.tile([128, C], I32, tag="ki")
                kf = scr_pool.tile([128, C], F16, tag="kf")
                nc.gpsimd.tensor_copy(ki[:], PI[:])
                nc.gpsimd.tensor_copy(kf[:], ki[:])

                _ladder(nc, X, BB, WW, UU, TI[:], True, 0, 0, bias_t[:, 0:1],
                        bias_t[:, 2:3])
                _ladder(nc, X, BB, WW, UU, kf[:], False, 27, R,
                        bias_t[:, 0:1], bias_t[:, 2:3])

                # per-batch Grams: accumulate CH chunks into PSUM
                ps = psum_pool.tile([WR, SL * UR], F32, tag="gram")
                for b in range(SL):
                    for cc in range(CH):
                        j = b * CH + cc
                        nc.tensor.matmul(ps[:, b * UR:(b + 1) * UR],
                                         WW[:, :, j], UU[:, :, j],
                                         start=(cc == 0), stop=(cc == CH - 1))
                gs = scr_pool.tile([WR, SL * UR], F16, tag="gs")
                nc.vector.scalar_tensor_tensor(gs[:], ps[:], 1.0, gct[:],
                                               ALU.mult, ALU.mult)
                ps2 = psc_pool.tile([1, SL * UR], F32, tag="colsum")
                nc.tensor.matmul(ps2[:], ones55[:], gs[:], start=True, stop=True)
                sall = scr_pool.tile([1, SL * UR], F32, tag="sall")
                nc.scalar.copy(sall[:], ps2[:])
                nc.vector.tensor_reduce(
                    diffs[:, s * SL:(s + 1) * SL],
                    sall[:].rearrange("p (b n) -> p b n", n=UR),
                    mybir.AxisListType.X, ALU.add)

            # final: v0 = sum_b diffs^2 ; penalty partition sums
            dsq = small_pool.tile([1, BC], F32)
            nc.scalar.activation(dsq[:], diffs[:], AFT.Square)
            v0 = small_pool.tile([1, 1], F32)
            nc.vector.tensor_reduce(v0[:], dsq[:], mybir.AxisListType.X, ALU.add)
            pen_red = small_pool.tile([128, 4], F16)
            with nc.allow_low_precision(reason="penalty sums are O(10) scalars"):
                for idx, t in enumerate((pen_i, pen_a, pen_m, pen_r)):
                    nc.vector.tensor_reduce(pen_red[:, idx:idx + 1], t[:],
                                            mybir.AxisListType.X, ALU.add)
            ps3 = psc_pool.tile([1, 4], F32, tag="pen")
            nc.tensor.matmul(ps3[:], ones128[:], pen_red[:], start=True, stop=True)
            ot = small_pool.tile([1, 8], F32)
            nc.vector.tensor_copy(ot[:, 0:1], v0[:])
            nc.vector.tensor_copy(ot[:, 1:5], ps3[:])
            nc.gpsimd.memset(ot[:, 5:8], 0.0)
            nc.sync.dma_start(out.ap(), ot[:])
    nc.compile()
    return nc


@functools.lru_cache(maxsize=2)
def _compiled():
    return _build()


def kernel(y_pred: np.ndarray, y: np.ndarray, calories_coeffs: np.ndarray,
           _trace: bool = False):
    G = _solve_G(np.asarray(calories_coeffs, np.float64)) / 700.0
    gcv = np.zeros((WR, SL * UR), np.float32)
    for b in range(SL):
        blk = gcv[:, b * UR:(b + 1) * UR]
        blk[0, 0:R] = G[0]
        blk[1:A, 0:R] = G[1:]
        blk[0, R:2 * R] = -G[0]
        blk[A:WR, R:2 * R] = -G[1:]

    yp = np.asarray(y_pred, np.float32).reshape(B, J, 2)
    yt = np.asarray(y, np.float32).reshape(B, J, 2)
    ip_h = np.ascontiguousarray(yp[:, :, 0])
    pa_h = np.ascontiguousarray(yp[:, :, 1].astype(np.float16))
    it_h = np.ascontiguousarray(yt[:, :, 0])
    ta_h = np.ascontiguousarray(yt[:, :, 1].astype(np.float16))
    in_maps = []
    for i in range(N_CORES):
        sl_ = slice(i * BC, (i + 1) * BC)
        in_maps.append({"ip": ip_h[sl_], "pa": pa_h[sl_], "it": it_h[sl_],
                        "ta": ta_h[sl_], "gc": gcv})
    nc = _compiled()
    res = run_bass_kernel_spmd(nc, in_maps, list(range(N_CORES)), trace=_trace)
    parts = np.stack([r["out"][0] for r in res.results])  # [8, 8]
    tot = parts.sum(axis=0).astype(np.float64)
    v0, a1, a2, a3, rl = tot[0], tot[1], tot[2], tot[3], tot[4]
    loss = (v0 + (a1 + a2 - 2.0 * a3) + rl) / float(B)
    outv = np.float32(loss)
    if _trace:
        return outv, res
    return outv
